# revision 19
# baseline (speedup 1.0000x reference)
"""Int8RouterLinear TRN2 kernel: out[16384, 64] = x[16384, 4096] @ (W_int8 * scale)^T.

Strategy (data-parallel over 8 NeuronCores, 2048 tokens each):
  - x (f32) is viewed as pairs of u16; the hi-u16 of each f32 IS bf16(x)
    (truncated). The XBAR DMA-transpose (2-byte dtypes, DRAM->SBUF) loads
    x^T directly into SBUF: xt[p, g, t] = x_u16[t, g*128 + p]. Odd
    partitions hold bf16(x) for h = 64g + (p-1)/2; even partitions hold
    the lo-u16 garbage (can be NaN-patterned).
  - A DVE copy_predicated pass overwrites even (garbage) partitions with
    1.0 (mask doubles as data); the stationary weights have 0.0 there, so
    garbage contributes exactly 0 to the contraction.
  - The router weight is tiny: host dequantizes + lays it out as
    w[p, g, e] = bf16(W^T[64g + (p-1)/2, e]) on odd p, 0.0 on even p.
  - PE then runs ONLY regular bf16 matmuls (64 accumulating MMs per
    256-token slab, moving N=256) - no PE transposes, no PSUM round-trip
    for x, and the dense MM stream keeps the HAM clock-gate warm.
  - All XBAR transposes are serialized on the sync queue (concurrent
    XBAR transposes hard-wedge the device); output DMAs interleave on the
    same queue two slabs behind, so nothing ever runs beside the XBAR.
  - out^T chunks DMA to DRAM as [64, 2048]; host transposes/concats (4MB).
"""
import numpy as np
import ml_dtypes

import concourse.mybir as mybir
from concourse import bacc
from concourse.tile import TileContext
from concourse.bass_utils import run_bass_kernel_spmd

TOKENS = 16384
HIDDEN = 4096
EXPERTS = 64
NCORES = 8
TSHARD = TOKENS // NCORES          # 2048 tokens per core
C = 2 * HIDDEN                     # 8192 u16 columns
G = C // 128                       # 64 column-groups per transpose
SLABS = [512, 512, 512, 256, 256]  # tokens per XBAR transpose (descending
                                   # sizes shrink the post-DMA pipeline tail)
SLABMAX = max(SLABS)
FIXG = 8                           # g-groups per garbage-fix call

BF16 = mybir.dt.bfloat16
F32 = mybir.dt.float32

_cache = {}


def _build():
    if "nc" in _cache:
        return _cache["nc"]
    nc = bacc.Bacc("TRN2", target_bir_lowering=False, debug=False,
                   num_devices=NCORES)
    x_d = nc.dram_tensor("x", [TSHARD, C], BF16, kind="ExternalInput")
    w_d = nc.dram_tensor("w", [128, G * EXPERTS], BF16, kind="ExternalInput")
    m_d = nc.dram_tensor("m", [128, FIXG * SLABMAX], mybir.dt.uint16,
                         kind="ExternalInput")
    o_d = nc.dram_tensor("out", [EXPERTS, TSHARD], F32, kind="ExternalOutput")
    x = x_d.ap()
    o = o_d.ap()

    with TileContext(nc) as tc:
        with tc.tile_pool(name="consts", bufs=1) as cpool, \
             tc.tile_pool(name="xt", bufs=2) as xtpool, \
             tc.tile_pool(name="ps", bufs=2, space="PSUM") as pspool, \
             tc.tile_pool(name="ot", bufs=2) as otpool:
            w_sb = cpool.tile([128, G, EXPERTS], BF16)
            nc.sync.dma_start(out=w_sb, in_=w_d.ap())
            mask = cpool.tile([128, FIXG, SLABMAX], mybir.dt.uint16)
            nc.sync.dma_start(out=mask, in_=m_d.ap())

            outs = []

            def drain_out(upto):
                while len(outs) > upto:
                    t0, slab0, ot0 = outs.pop(0)
                    nc.sync.dma_start(out=o[:, t0:t0 + slab0], in_=ot0)

            tok = 0
            for s, slab in enumerate(SLABS):
                xt = xtpool.tile([128, G, SLABMAX], BF16, name="xt", tag="xt")
                nc.sync.dma_start_transpose(
                    xt[:, :, 0:slab], x[tok:tok + slab, :])
                # keep out-DMAs 2 slabs behind so they never stall the queue
                drain_out(2)

                # 4 concurrent K=32 strip-matmuls per g (tile_position row
                # tiling), each strip accumulating into its own PSUM bank
                po = pspool.tile([EXPERTS, 4, SLABMAX], F32, name="po",
                                 tag="po")
                for j in range(G // FIXG):
                    xv = xt[:, j * FIXG:(j + 1) * FIXG, 0:slab].bitcast(
                        mybir.dt.uint16)
                    nc.vector.tensor_tensor(
                        out=xv, in0=xv, in1=mask[:, :, 0:slab],
                        op=mybir.AluOpType.bitwise_and)
                    for g in range(j * FIXG, (j + 1) * FIXG):
                        for i in range(4):
                            nc.tensor.matmul(
                                po[:, i, 0:slab],
                                w_sb[32 * i:32 * i + 32, g, :],
                                xt[32 * i:32 * i + 32, g, 0:slab],
                                start=(g == 0), stop=(g == G - 1),
                                tile_position=(32 * i, 0))
                ot = otpool.tile([EXPERTS, SLABMAX], F32, name="ot", tag="ot")
                nc.vector.tensor_copy(ot[:, 0:slab], po[:, 0, 0:slab])
                for i in range(1, 4):
                    nc.vector.tensor_add(ot[:, 0:slab], ot[:, 0:slab],
                                         po[:, i, 0:slab])
                outs.append((tok, slab, ot[:, 0:slab]))
                tok += slab
            drain_out(0)

    nc.compile()
    _cache["nc"] = nc
    return nc


def _prep_w(weights_int8, scales):
    wt = (weights_int8.astype(np.float32)
          * scales.astype(np.float32)[:, None]).T   # [HIDDEN, EXPERTS]
    warr = np.zeros((128, G, EXPERTS), dtype=ml_dtypes.bfloat16)
    warr[1::2, :, :] = wt.reshape(G, 64, EXPERTS).transpose(1, 0, 2)
    return np.ascontiguousarray(warr.reshape(128, G * EXPERTS))


def _prep_mask():
    m = np.zeros((128, FIXG * SLABMAX), dtype=np.uint16)
    m[1::2, :] = 0xFFFF          # odd partitions hold real data - keep
    return m


def _make_inmaps(x, weights_int8, scales):
    warr = _prep_w(weights_int8, scales)
    marr = _prep_mask()
    xu = np.ascontiguousarray(x, dtype=np.float32).view(np.uint16)
    xv = xu.reshape(TOKENS, C).view(ml_dtypes.bfloat16)
    return [
        {"x": xv[c * TSHARD:(c + 1) * TSHARD], "w": warr, "m": marr}
        for c in range(NCORES)
    ]


def kernel(x, weights_int8, scales):
    nc = _build()
    in_maps = _make_inmaps(x, weights_int8, scales)
    res = run_bass_kernel_spmd(nc, in_maps, core_ids=list(range(NCORES)))
    out = np.concatenate(
        [np.asarray(res.results[c]["out"]).T for c in range(NCORES)], axis=0)
    return np.ascontiguousarray(out, dtype=np.float32)


# revision 22
# speedup vs baseline: 1.0100x; 1.0100x over previous
"""Int8RouterLinear TRN2 kernel: out[16384, 64] = x[16384, 4096] @ (W_int8 * scale)^T.

Strategy (data-parallel over 8 NeuronCores, 2048 tokens each):
  - x (f32) is viewed as pairs of u16; the hi-u16 of each f32 IS bf16(x)
    (truncated). The XBAR DMA-transpose (2-byte dtypes, DRAM->SBUF) loads
    x^T directly into SBUF: xt[p, g, t] = x_u16[t, g*128 + p]. Odd
    partitions hold bf16(x) for h = 64g + (p-1)/2; even partitions hold
    the lo-u16 garbage (can be NaN-patterned).
  - A DVE copy_predicated pass overwrites even (garbage) partitions with
    1.0 (mask doubles as data); the stationary weights have 0.0 there, so
    garbage contributes exactly 0 to the contraction.
  - The router weight is tiny: host dequantizes + lays it out as
    w[p, g, e] = bf16(W^T[64g + (p-1)/2, e]) on odd p, 0.0 on even p.
  - PE then runs ONLY regular bf16 matmuls (64 accumulating MMs per
    256-token slab, moving N=256) - no PE transposes, no PSUM round-trip
    for x, and the dense MM stream keeps the HAM clock-gate warm.
  - All XBAR transposes are serialized on the sync queue (concurrent
    XBAR transposes hard-wedge the device); output DMAs interleave on the
    same queue two slabs behind, so nothing ever runs beside the XBAR.
  - out^T chunks DMA to DRAM as [64, 2048]; host transposes/concats (4MB).
"""
import numpy as np
import ml_dtypes

import concourse.mybir as mybir
from concourse import bacc
from concourse.tile import TileContext
from concourse.bass_utils import run_bass_kernel_spmd

TOKENS = 16384
HIDDEN = 4096
EXPERTS = 64
NCORES = 8
TSHARD = TOKENS // NCORES          # 2048 tokens per core
C = 2 * HIDDEN                     # 8192 u16 columns
G = C // 128                       # 64 column-groups per transpose
SLABS = [512, 512, 512, 256, 128, 128]   # tokens per XBAR transpose
                                   # (descending sizes shrink the pipeline tail)
SLABMAX = max(SLABS)
FIXG = 8                           # g-groups per garbage-fix call

BF16 = mybir.dt.bfloat16
F32 = mybir.dt.float32

_cache = {}


def _build():
    if "nc" in _cache:
        return _cache["nc"]
    nc = bacc.Bacc("TRN2", target_bir_lowering=False, debug=False,
                   num_devices=NCORES)
    x_d = nc.dram_tensor("x", [TSHARD, C], BF16, kind="ExternalInput")
    w_d = nc.dram_tensor("w", [128, G * EXPERTS], BF16, kind="ExternalInput")
    m_d = nc.dram_tensor("m", [128, FIXG * SLABMAX], mybir.dt.uint16,
                         kind="ExternalInput")
    o_d = nc.dram_tensor("out", [EXPERTS, TSHARD], F32, kind="ExternalOutput")
    x = x_d.ap()
    o = o_d.ap()

    with TileContext(nc) as tc:
        with tc.tile_pool(name="consts", bufs=1) as cpool, \
             tc.tile_pool(name="xt", bufs=2) as xtpool, \
             tc.tile_pool(name="ps", bufs=2, space="PSUM") as pspool, \
             tc.tile_pool(name="ot", bufs=2) as otpool:
            w_sb = cpool.tile([128, G, EXPERTS], BF16)
            mask = cpool.tile([128, FIXG, SLABMAX], mybir.dt.uint16)

            outs = []

            def drain_out(upto):
                while len(outs) > upto:
                    t0, slab0, ot0 = outs.pop(0)
                    nc.sync.dma_start(out=o[:, t0:t0 + slab0], in_=ot0)

            tok = 0
            for s, slab in enumerate(SLABS):
                xt = xtpool.tile([128, G, SLABMAX], BF16, name="xt", tag="xt")
                nc.sync.dma_start_transpose(
                    xt[:, :, 0:slab], x[tok:tok + slab, :])
                if s == 0:
                    # constants load AFTER the first transpose on the queue:
                    # T1 starts ~7us earlier, consumers absorb the delay
                    nc.sync.dma_start(out=w_sb, in_=w_d.ap())
                    nc.sync.dma_start(out=mask, in_=m_d.ap())
                # keep out-DMAs 2 slabs behind so they never stall the queue
                drain_out(2)

                po = pspool.tile([EXPERTS, SLABMAX], F32, name="po", tag="po")
                for j in range(G // FIXG):
                    xv = xt[:, j * FIXG:(j + 1) * FIXG, 0:slab].bitcast(
                        mybir.dt.uint16)
                    nc.vector.tensor_tensor(
                        out=xv, in0=xv, in1=mask[:, :, 0:slab],
                        op=mybir.AluOpType.bitwise_and)
                    for g in range(j * FIXG, (j + 1) * FIXG):
                        nc.tensor.matmul(po[:, 0:slab], w_sb[:, g, :],
                                         xt[:, g, 0:slab],
                                         start=(g == 0), stop=(g == G - 1))
                ot = otpool.tile([EXPERTS, SLABMAX], F32, name="ot", tag="ot")
                nc.vector.tensor_copy(ot[:, 0:slab], po[:, 0:slab])
                outs.append((tok, slab, ot[:, 0:slab]))
                tok += slab
            drain_out(0)

    nc.compile()
    _cache["nc"] = nc
    return nc


def _prep_w(weights_int8, scales):
    wt = (weights_int8.astype(np.float32)
          * scales.astype(np.float32)[:, None]).T   # [HIDDEN, EXPERTS]
    warr = np.zeros((128, G, EXPERTS), dtype=ml_dtypes.bfloat16)
    warr[1::2, :, :] = wt.reshape(G, 64, EXPERTS).transpose(1, 0, 2)
    return np.ascontiguousarray(warr.reshape(128, G * EXPERTS))


def _prep_mask():
    m = np.zeros((128, FIXG * SLABMAX), dtype=np.uint16)
    m[1::2, :] = 0xFFFF          # odd partitions hold real data - keep
    return m


def _make_inmaps(x, weights_int8, scales):
    warr = _prep_w(weights_int8, scales)
    marr = _prep_mask()
    xu = np.ascontiguousarray(x, dtype=np.float32).view(np.uint16)
    xv = xu.reshape(TOKENS, C).view(ml_dtypes.bfloat16)
    return [
        {"x": xv[c * TSHARD:(c + 1) * TSHARD], "w": warr, "m": marr}
        for c in range(NCORES)
    ]


def kernel(x, weights_int8, scales):
    nc = _build()
    in_maps = _make_inmaps(x, weights_int8, scales)
    res = run_bass_kernel_spmd(nc, in_maps, core_ids=list(range(NCORES)))
    out = np.concatenate(
        [np.asarray(res.results[c]["out"]).T for c in range(NCORES)], axis=0)
    return np.ascontiguousarray(out, dtype=np.float32)
